# revision 1
# baseline (speedup 1.0000x reference)
"""DeepSeek-V3 token-choice top-k router on 8 Trainium2 NeuronCores.

Strategy (per core, data-parallel over tokens; 1024 tokens/core):
  - x shard [1024, 7168] fp32 streamed from HBM in 8 token-tiles of [128, 7168].
  - Gate weight split on host into fp32r hi/lo pair (exact: hi + lo == w in
    fp32) and packed to [128, 56*256] d-major chunks, replicated per core.
  - PE: per 128-token tile, transpose x chunks ([128t,128d] -> [128d,128t]);
    ACT casts the PSUM transpose to fp32r (hi, round-to-nearest); DVE computes
    lo = x_T - hi (exact Sterbenz subtract, cast fp32r keeps all useful bits).
    Then 3 accumulating fp32r matmuls per chunk (hi@w_hi + hi@w_lo + lo@w_hi;
    the dropped lo@w_lo term is ~2^-26 relative) -> exact-fp32-grade logits
    [128 tokens, 256 experts] in PSUM at 1 cycle/row instead of fp32's 4.
  - ACT: sigmoid(logits) PSUM->SBUF.
  - DVE: hardware top-8 (`max`/`max_index`) for group top-2 sums, top-4 group
    threshold, masked top-8; normalization.
  - GPSIMD: bias add, group masking, and the one-hot weight gathers
    (scalar_tensor_tensor with accumulate), keeping DVE under the PE span.
"""

import numpy as np

N = 8192
D = 7168
E = 256
G = 8
EPG = E // G  # 32
TOPK_GROUP = 4
TOP_K = 8
SCALING = 2.5
N_CORES = 8
NPC = N // N_CORES  # 1024 tokens per core
P = 128
KC = D // P  # 56 contraction chunks
TT = NPC // P  # 8 token tiles per core
KB = 4  # k-chunks per transpose batch (one PSUM bank)
NB = KC // KB  # 14 batches

_CACHE = {}


def build_program(mode="f32r_3pass"):
    import concourse.bacc as bacc
    import concourse.mybir as mybir
    from concourse import tile, masks

    nc = bacc.Bacc(
        "TRN2",
        target_bir_lowering=False,
        debug=False,
        enable_asserts=True,
        num_devices=N_CORES,
    )
    f32 = mybir.dt.float32
    f32r = mybir.dt.float32r
    i32 = mybir.dt.int32
    u32 = mybir.dt.uint32
    AF = mybir.ActivationFunctionType
    OP = mybir.AluOpType
    AX = mybir.AxisListType

    three_pass = mode == "f32r_3pass"
    gdt = f32r if three_pass else f32

    x_d = nc.dram_tensor("x", [NPC, D], f32, kind="ExternalInput").ap()
    if three_pass:
        gwh_d = nc.dram_tensor("gw2", [P, KC * 2 * E], f32r, kind="ExternalInput").ap()
    else:
        gwh_d = nc.dram_tensor("gwh", [P, KC * E], gdt, kind="ExternalInput").ap()
    bias_d = nc.dram_tensor("bias", [1, E], f32, kind="ExternalInput").ap()
    idx_d = nc.dram_tensor("idx", [NPC, TOP_K], i32, kind="ExternalOutput").ap()
    w_d = nc.dram_tensor("w", [NPC, TOP_K], f32, kind="ExternalOutput").ap()

    with tile.TileContext(nc) as tc:
        with (
            tc.tile_pool(name="const", bufs=1) as const_pool,
            tc.tile_pool(name="gw", bufs=1) as gw_pool,
            tc.tile_pool(name="x", bufs=2) as x_pool,
            tc.tile_pool(name="xt", bufs=4) as xt_pool,
            tc.tile_pool(name="ptr", bufs=3, space="PSUM") as ptr_pool,
            tc.tile_pool(name="plog", bufs=2, space="PSUM") as plog_pool,
            tc.tile_pool(name="work", bufs=2) as work_pool,
            tc.tile_pool(name="outs", bufs=2) as out_pool,
        ):
            # ---- tiny bias DMA first, then first x tile, then gw quarters ----
            bias_sb = const_pool.tile([1, E], f32, name="biassb")
            nc.sync.dma_start(bias_sb[:], bias_d[:])
            x_tiles = {}
            x_tiles[0] = x_pool.tile([P, D], f32, tag="xtile", name="xtile0")
            nc.sync.dma_start(x_tiles[0][:], x_d[0:P, :])

            # ---- gate weight DMA in quarters; x1 issued between q1 and q2
            if three_pass:
                gwh_sb = gw_pool.tile([P, KC * 2 * E], f32r, name="gw2sb")
                q = KC * 2 * E // 4
                nc.sync.dma_start(gwh_sb[:, 0 * q : 1 * q], gwh_d[:, 0 * q : 1 * q])
                nc.sync.dma_start(gwh_sb[:, 1 * q : 2 * q], gwh_d[:, 1 * q : 2 * q])
                x_tiles[1] = x_pool.tile([P, D], f32, tag="xtile", name="xtile1")
                nc.sync.dma_start(x_tiles[1][:], x_d[P : 2 * P, :])
                nc.sync.dma_start(gwh_sb[:, 2 * q : 3 * q], gwh_d[:, 2 * q : 3 * q])
                nc.sync.dma_start(gwh_sb[:, 3 * q : 4 * q], gwh_d[:, 3 * q : 4 * q])
                gwh_v = gwh_sb[:].rearrange("p (k e) -> p k e", k=KC)
            else:
                gwh_sb = gw_pool.tile([P, KC * E], gdt, name="gwhsb")
                nc.sync.dma_start(gwh_sb[:], gwh_d[:])
                gwh_v = gwh_sb[:].rearrange("p (k e) -> p k e", k=KC)

            # ---- constants ----
            ident = const_pool.tile([P, P], f32)
            masks.make_identity(nc, ident[:])
            iota_i = const_pool.tile([P, E], i32)
            nc.gpsimd.iota(iota_i[:], pattern=[[1, E]], base=0, channel_multiplier=0)
            iota_f = const_pool.tile([P, E], f32)
            nc.vector.tensor_copy(iota_f[:], iota_i[:])
            bias_rep = const_pool.tile([P, E], f32)
            nc.gpsimd.partition_broadcast(bias_rep[:], bias_sb[0:1, :])

            for t in range(TT):
                if t not in x_tiles:
                    x_tiles[t] = x_pool.tile([P, D], f32, tag="xtile", name=f"xtile{t}")
                    nc.sync.dma_start(x_tiles[t][:], x_d[t * P : (t + 1) * P, :])
                if t + 1 < TT and (t + 1) not in x_tiles:
                    x_tiles[t + 1] = x_pool.tile([P, D], f32, tag="xtile", name=f"xtile{t+1}")
                    nc.sync.dma_start(
                        x_tiles[t + 1][:], x_d[(t + 1) * P : (t + 2) * P, :]
                    )
                x_tile = x_tiles[t]

                if three_pass:
                    plog = plog_pool.tile([P, 2 * E], f32, tag="plog")
                else:
                    plog = plog_pool.tile([P, E], f32, tag="plog")
                for b in range(NB):
                    ptr = ptr_pool.tile([P, KB * P], f32, tag="ptr")
                    for j in range(KB):
                        k = b * KB + j
                        nc.tensor.matmul(
                            ptr[:, j * P : (j + 1) * P],
                            x_tile[:, k * P : (k + 1) * P],
                            ident[:],
                            is_transpose=True,
                        )
                    if three_pass:
                        hi_sb = xt_pool.tile([P, KB * P], f32r, tag="hi")
                        nc.scalar.copy(hi_sb[:], ptr[:])
                        lo_sb = xt_pool.tile([P, KB * P], f32r, tag="lo")
                        nc.vector.scalar_tensor_tensor(
                            lo_sb[:], ptr[:], 0.0, hi_sb[:].bitcast(f32),
                            op0=OP.add, op1=OP.subtract,
                        )
                        for j in range(KB):
                            k = b * KB + j
                            sl = slice(j * P, (j + 1) * P)
                            nc.tensor.matmul(
                                plog[:], hi_sb[:, sl], gwh_v[:, k, :],
                                start=(k == 0), stop=False,
                            )
                            nc.tensor.matmul(
                                plog[:, 0:E], lo_sb[:, sl], gwh_v[:, k, 0:E],
                                start=False, stop=(k == KC - 1),
                            )
                    else:
                        xt_sb = xt_pool.tile([P, KB * P], f32, tag="hi")
                        nc.scalar.copy(xt_sb[:], ptr[:])
                        for j in range(KB):
                            k = b * KB + j
                            nc.tensor.matmul(
                                plog[:],
                                xt_sb[:, j * P : (j + 1) * P],
                                gwh_v[:, k, :],
                                start=(k == 0), stop=(k == KC - 1),
                            )

                # ---- routing for this token tile ----
                scores = work_pool.tile([P, E], f32, tag="scores")
                if three_pass:
                    half2 = work_pool.tile([P, E], f32, tag="half2")
                    nc.scalar.copy(half2[:], plog[:, E : 2 * E])
                    lsum = work_pool.tile([P, E], f32, tag="lsum")
                    nc.vector.tensor_tensor(
                        lsum[:], plog[:, 0:E], half2[:], op=OP.add
                    )
                    nc.scalar.activation(scores[:], lsum[:], AF.Sigmoid)
                else:
                    nc.scalar.activation(scores[:], plog[:], AF.Sigmoid)

                sfc = work_pool.tile([P, E], f32, tag="sfc")
                nc.gpsimd.tensor_tensor(sfc[:], scores[:], bias_rep[:], op=OP.add)

                # per-group top-8 (need top-2 of each group of 32)
                gtops = work_pool.tile([P, G * 8], f32, tag="gtops")
                for g in range(G):
                    nc.vector.max(
                        gtops[:, g * 8 : (g + 1) * 8],
                        sfc[:, g * EPG : (g + 1) * EPG],
                    )
                gv = gtops[:].rearrange("p (g k) -> p g k", g=G)
                gs = work_pool.tile([P, G], f32, tag="gs")
                nc.vector.tensor_tensor(gs[:], gv[:, :, 0], gv[:, :, 1], op=OP.add)

                # top-4 groups -> mask
                gtop8 = work_pool.tile([P, 8], f32, tag="gtop8")
                nc.vector.max(gtop8[:], gs[:])
                gmask = work_pool.tile([P, G], f32, tag="gmask")
                nc.vector.tensor_scalar(
                    gmask[:], gs[:], gtop8[:, TOPK_GROUP - 1 : TOPK_GROUP], None,
                    op0=OP.is_ge,
                )

                # masked scores
                tmp = work_pool.tile([P, E], f32, tag="tmp")
                for g in range(G):
                    nc.vector.tensor_scalar(
                        tmp[:, g * EPG : (g + 1) * EPG],
                        sfc[:, g * EPG : (g + 1) * EPG],
                        gmask[:, g : g + 1],
                        None,
                        op0=OP.mult,
                    )

                # top-8 values + indices
                vals = work_pool.tile([P, TOP_K], f32, tag="vals")
                nc.vector.max(vals[:], tmp[:])
                idxu = work_pool.tile([P, TOP_K], u32, tag="idxu")
                nc.vector.max_index(idxu[:], vals[:], tmp[:])
                idxf = work_pool.tile([P, TOP_K], f32, tag="idxf")
                nc.vector.tensor_copy(idxf[:], idxu[:])

                # gather raw sigmoid scores at the selected indices (GPSIMD)
                w8 = out_pool.tile([P, TOP_K], f32, tag="w8")
                scratch = work_pool.tile([P, E], f32, tag="scratch")
                for j in range(TOP_K):
                    nc.vector.scalar_tensor_tensor(
                        scratch[:],
                        iota_f[:],
                        idxf[:, j : j + 1],
                        scores[:],
                        op0=OP.is_equal,
                        op1=OP.mult,
                        accum_out=w8[:, j : j + 1],
                    )

                # normalize + scale
                wsum = work_pool.tile([P, 1], f32, tag="wsum")
                nc.vector.reduce_sum(wsum[:], w8[:], axis=AX.X)
                wse = work_pool.tile([P, 1], f32, tag="wse")
                nc.vector.tensor_scalar(wse[:], wsum[:], 1e-20, None, op0=OP.add)
                wrec = work_pool.tile([P, 1], f32, tag="wrec")
                nc.vector.reciprocal(wrec[:], wse[:])
                w_out = out_pool.tile([P, TOP_K], f32, tag="wout")
                nc.vector.tensor_scalar(
                    w_out[:], w8[:], wrec[:, 0:1], float(SCALING),
                    op0=OP.mult, op1=OP.mult,
                )
                idx_out = out_pool.tile([P, TOP_K], i32, tag="idxout")
                nc.vector.tensor_copy(idx_out[:], idxu[:])

                nc.sync.dma_start(idx_d[t * P : (t + 1) * P, :], idx_out[:])
                nc.sync.dma_start(w_d[t * P : (t + 1) * P, :], w_out[:])

    nc.compile()
    return nc


def _get_nc(**kw):
    key = tuple(sorted(kw.items()))
    if key not in _CACHE:
        _CACHE[key] = build_program(**kw)
    return _CACHE[key]


def _fp32r_round(a):
    # round-to-nearest fp32 -> fp32r (12-bit mantissa), bit-exact with HW cast
    bits = np.ascontiguousarray(a).view(np.uint32)
    keep = np.uint32(0xFFFFF000)
    rounded = (bits + np.uint32(0x800)) & keep  # round-half-up into kept bits
    # correct round-to-nearest-even on the halfway case
    half = (bits & np.uint32(0xFFF)) == np.uint32(0x800)
    even = ((bits >> np.uint32(12)) & np.uint32(1)) == 0
    rounded = np.where(half & even, bits & keep, rounded)
    return rounded.view(np.float32).reshape(a.shape)


def _pack(a2d):
    # [D, E] -> [P, KC*E]: partition p holds rows k*128+p
    return np.ascontiguousarray(
        a2d.reshape(KC, P, E).transpose(1, 0, 2)
    ).reshape(P, KC * E)


def _run(x, gate_w, bias, trace=False, **build_kw):
    from concourse.bass_utils import run_bass_kernel_spmd

    x = np.ascontiguousarray(np.asarray(x, dtype=np.float32))
    gate_w = np.ascontiguousarray(np.asarray(gate_w, dtype=np.float32))
    bias = np.ascontiguousarray(np.asarray(bias, dtype=np.float32))
    nc = _get_nc(**build_kw)
    mode = build_kw.get("mode", "f32r_3pass")
    gwt = np.ascontiguousarray(gate_w.T)  # [D, E]
    bias2d = bias.reshape(1, E)
    if mode == "f32r_3pass":
        gw_hi = _fp32r_round(gwt)
        gw_lo = _fp32r_round(gwt - gw_hi)
        ph = _pack(gw_hi).reshape(P, KC, E)
        pl = _pack(gw_lo).reshape(P, KC, E)
        gw2 = np.concatenate([ph, pl], axis=2).reshape(P, KC * 2 * E)
        maps = {"gw2": np.ascontiguousarray(gw2), "bias": bias2d}
    else:
        maps = {"gwh": _pack(gwt), "bias": bias2d}
    in_maps = [
        {"x": x[c * NPC : (c + 1) * NPC], **maps} for c in range(N_CORES)
    ]
    res = run_bass_kernel_spmd(nc, in_maps, core_ids=list(range(N_CORES)), trace=trace)
    idx = np.concatenate([res.results[c]["idx"] for c in range(N_CORES)], axis=0)
    w = np.concatenate([res.results[c]["w"] for c in range(N_CORES)], axis=0)
    return (idx.astype(np.int32), w.astype(np.float32)), res


def kernel(x, gate_w, bias):
    (idx, w), _ = _run(x, gate_w, bias)
    return idx, w



# revision 2
# speedup vs baseline: 1.3108x; 1.3108x over previous
"""DeepSeek-V3 token-choice top-k router on 8 Trainium2 NeuronCores.

Strategy (per core, data-parallel over tokens; 1024 tokens/core):
  - Host pre-transposes x to d-major and packs per k-chunk, so no PE
    transposes are needed: per chunk k the PE computes
    logitsT[e, t] += W_k[d, e].T @ x_k[d, t] with W as the stationary
    operand and tokens streaming (N = batch size columns).
  - Exact fp32 -grade logits via 3 fp32r passes: x split on device into
    hi + lo (exact Sterbenz split, ACT cast + DVE subtract), W split on
    host into hi + lo fp32r.  Accumulated terms: xh@Wh + xl@Wh + xh@Wl
    (dropped xl@Wl is ~2^-26 relative).
  - Tokens processed in batches [512, 256, 256]; each batch's PSUM
    logitsT ([128e, TB] x 2 expert halves) -> ACT sigmoid -> PE
    transpose back to token-major [128t, 256e] -> DVE/ACT routing
    (group top-2 sums, top-4 groups, masked top-8, one-hot gathers,
    normalize).  A batch's routing is emitted interleaved into the next
    batch's chunk loop so only the last (256-token) batch's routing is
    an exposed tail.
  - W DMAs ride the Activation HWDGE queue, the x stream rides the Sync
    queue (2-chunk transfers, ring prefetch).
"""

import numpy as np

N = 8192
D = 7168
E = 256
G = 8
EPG = E // G  # 32
TOPK_GROUP = 4
TOP_K = 8
SCALING = 2.5
N_CORES = 8
NPC = N // N_CORES  # 1024 tokens per core
P = 128
KC = D // P  # 56 contraction chunks
BATCHES = [512, 256, 256]
KU = 2  # k-chunks per DMA/split unit
NKU = KC // KU  # 28 units

_CACHE = {}


def build_program():
    import concourse.bacc as bacc
    import concourse.mybir as mybir
    from concourse import tile, masks

    nc = bacc.Bacc(
        "TRN2",
        target_bir_lowering=False,
        debug=False,
        enable_asserts=True,
        num_devices=N_CORES,
    )
    f32 = mybir.dt.float32
    f32r = mybir.dt.float32r
    i32 = mybir.dt.int32
    u32 = mybir.dt.uint32
    AF = mybir.ActivationFunctionType
    OP = mybir.AluOpType
    AX = mybir.AxisListType

    XCOLS = KC * NPC  # 57344
    x_d = nc.dram_tensor("x", [P, XCOLS], f32, kind="ExternalInput").ap()
    gw_d = nc.dram_tensor("gw", [P, KC * 4 * P], f32r, kind="ExternalInput").ap()
    bias_d = nc.dram_tensor("bias", [1, E], f32, kind="ExternalInput").ap()
    idx_d = nc.dram_tensor("idx", [NPC, TOP_K], i32, kind="ExternalOutput").ap()
    w_d = nc.dram_tensor("w", [NPC, TOP_K], f32, kind="ExternalOutput").ap()

    # column offset of batch b in the packed x layout
    xoff = []
    o = 0
    for TB in BATCHES:
        xoff.append(o)
        o += KC * TB

    with tile.TileContext(nc) as tc:
        with (
            tc.tile_pool(name="const", bufs=1) as const_pool,
            tc.tile_pool(name="gw", bufs=NKU) as gw_pool,
            tc.tile_pool(name="x", bufs=6) as x_pool,
            tc.tile_pool(name="xs", bufs=4) as xs_pool,
            tc.tile_pool(name="plog", bufs=4, space="PSUM") as plog_pool,
            tc.tile_pool(name="psc", bufs=4, space="PSUM") as psc_pool,
            tc.tile_pool(name="st", bufs=4) as st_pool,
            tc.tile_pool(name="work", bufs=2) as work_pool,
            tc.tile_pool(name="outs", bufs=4) as out_pool,
        ):
            # ---- W on the ACT HWDGE queue, issued first so it streams
            # ---- ahead; bias + first x units on the sync queue.
            gw_tiles = []
            for kk in range(NKU):
                g = gw_pool.tile([P, KU * 4 * P], f32r, tag="gwt", name=f"gw{kk}")
                nc.scalar.dma_start(g[:], gw_d[:, kk * KU * 4 * P : (kk + 1) * KU * 4 * P])
                gw_tiles.append(g)

            bias_sb = const_pool.tile([1, E], f32, name="biassb")
            nc.sync.dma_start(bias_sb[:], bias_d[:])

            # ---- constants ----
            ident = const_pool.tile([P, P], f32)
            masks.make_identity(nc, ident[:])
            iota_i = const_pool.tile([P, E], i32)
            nc.gpsimd.iota(iota_i[:], pattern=[[1, E]], base=0, channel_multiplier=0)
            iota_f = const_pool.tile([P, E], f32)
            nc.vector.tensor_copy(iota_f[:], iota_i[:])
            bias_rep = const_pool.tile([P, E], f32)
            nc.gpsimd.partition_broadcast(bias_rep[:], bias_sb[0:1, :])

            def routing_tile(sfcP, gt0):
                """Route one 128-token tile; sfcP = [128t, 256e] scores in PSUM."""
                sfc = work_pool.tile([P, E], f32, tag="sfc", bufs=3)
                nc.vector.tensor_tensor(sfc[:], sfcP[:], bias_rep[:], op=OP.add)
                scores = work_pool.tile([P, E], f32, tag="scores", bufs=3)
                nc.scalar.copy(scores[:], sfcP[:])

                # per-group top-8 (need top-2 of each group of 32)
                gtops = work_pool.tile([P, G * 8], f32, tag="gtops")
                for g in range(G):
                    nc.vector.max(
                        gtops[:, g * 8 : (g + 1) * 8],
                        sfc[:, g * EPG : (g + 1) * EPG],
                    )
                gv = gtops[:].rearrange("p (g k) -> p g k", g=G)
                gs = work_pool.tile([P, G], f32, tag="gs")
                nc.vector.tensor_tensor(gs[:], gv[:, :, 0], gv[:, :, 1], op=OP.add)

                # top-4 groups -> mask
                gtop8 = work_pool.tile([P, 8], f32, tag="gtop8")
                nc.vector.max(gtop8[:], gs[:])
                gmask = work_pool.tile([P, G], f32, tag="gmask")
                nc.vector.tensor_scalar(
                    gmask[:], gs[:], gtop8[:, TOPK_GROUP - 1 : TOPK_GROUP], None,
                    op0=OP.is_ge,
                )

                # masked scores
                tmp = work_pool.tile([P, E], f32, tag="tmp")
                for g in range(G):
                    nc.vector.tensor_scalar(
                        tmp[:, g * EPG : (g + 1) * EPG],
                        sfc[:, g * EPG : (g + 1) * EPG],
                        gmask[:, g : g + 1],
                        None,
                        op0=OP.mult,
                    )

                # top-8 values + indices
                vals = work_pool.tile([P, TOP_K], f32, tag="vals")
                nc.vector.max(vals[:], tmp[:])
                idxu = work_pool.tile([P, TOP_K], u32, tag="idxu")
                nc.vector.max_index(idxu[:], vals[:], tmp[:])
                idxf = work_pool.tile([P, TOP_K], f32, tag="idxf")
                nc.vector.tensor_copy(idxf[:], idxu[:])

                # gather raw sigmoid scores at the selected indices
                w8 = out_pool.tile([P, TOP_K], f32, tag="w8")
                scratch = work_pool.tile([P, E], f32, tag="scratch")
                for j in range(TOP_K):
                    nc.vector.scalar_tensor_tensor(
                        scratch[:],
                        iota_f[:],
                        idxf[:, j : j + 1],
                        scores[:],
                        op0=OP.is_equal,
                        op1=OP.mult,
                        accum_out=w8[:, j : j + 1],
                    )

                # normalize + scale
                wsum = work_pool.tile([P, 1], f32, tag="wsum")
                nc.vector.reduce_sum(wsum[:], w8[:], axis=AX.X)
                wse = work_pool.tile([P, 1], f32, tag="wse")
                nc.vector.tensor_scalar(wse[:], wsum[:], 1e-20, None, op0=OP.add)
                wrec = work_pool.tile([P, 1], f32, tag="wrec")
                nc.vector.reciprocal(wrec[:], wse[:])
                w_out = out_pool.tile([P, TOP_K], f32, tag="wout")
                nc.vector.tensor_scalar(
                    w_out[:], w8[:], wrec[:, 0:1], float(SCALING),
                    op0=OP.mult, op1=OP.mult,
                )
                idx_out = out_pool.tile([P, TOP_K], i32, tag="idxout")
                nc.vector.tensor_copy(idx_out[:], idxu[:])

                nc.sync.dma_start(idx_d[gt0 : gt0 + P, :], idx_out[:])
                nc.sync.dma_start(w_d[gt0 : gt0 + P, :], w_out[:])

            def epilogue_steps(b, plogs):
                """Closures: sigmoid+transposes, then one routing per tile."""
                TB = BATCHES[b]
                t0 = sum(BATCHES[:b])
                NT = TB // P
                state = {}

                def sig_and_transpose():
                    s0 = st_pool.tile([P, TB], f32, tag="sct", name=f"sct{b}0")
                    s1 = st_pool.tile([P, TB], f32, tag="sct", name=f"sct{b}1")
                    nc.scalar.activation(s0[:], plogs[0][:], AF.Sigmoid)
                    nc.scalar.activation(s1[:], plogs[1][:], AF.Sigmoid)
                    sfcPs = []
                    for tt in range(NT):
                        sp = psc_pool.tile([P, E], f32, tag="psc")
                        nc.tensor.matmul(
                            sp[:, 0:P], s0[:, tt * P : (tt + 1) * P], ident[:],
                            is_transpose=True,
                        )
                        nc.tensor.matmul(
                            sp[:, P:E], s1[:, tt * P : (tt + 1) * P], ident[:],
                            is_transpose=True,
                        )
                        sfcPs.append(sp)
                    state["sfcPs"] = sfcPs

                steps = [sig_and_transpose]
                for tt in range(NT):
                    def rt(tt=tt):
                        routing_tile(state["sfcPs"][tt], t0 + tt * P)
                    steps.append(rt)
                return steps

            pending = []  # epilogue closures of the previous batch

            for b, TB in enumerate(BATCHES):
                UC = KU * TB  # columns per x unit
                plogs = [
                    plog_pool.tile([P, TB], f32, tag="plog", name=f"plog{b}h{h}")
                    for h in range(2)
                ]
                for kk in range(NKU):
                    x_t = x_pool.tile([P, UC], f32, tag="xt", name=f"x{b}_{kk}")
                    nc.sync.dma_start(
                        x_t[:], x_d[:, xoff[b] + kk * UC : xoff[b] + (kk + 1) * UC]
                    )
                    xh = xs_pool.tile([P, UC], f32r, tag="xh")
                    nc.scalar.copy(xh[:], x_t[:])
                    xl = xs_pool.tile([P, UC], f32r, tag="xl")
                    nc.vector.scalar_tensor_tensor(
                        xl[:], x_t[:], 0.0, xh[:].bitcast(f32),
                        op0=OP.add, op1=OP.subtract,
                    )
                    g = gw_tiles[kk]
                    for j in range(KU):
                        k = kk * KU + j
                        wb = j * 4 * P
                        xs = slice(j * TB, (j + 1) * TB)
                        first = k == 0
                        last = k == KC - 1
                        # xh @ Wh (both halves)
                        nc.tensor.matmul(
                            plogs[0][:], g[:, wb : wb + P], xh[:, xs],
                            start=first, stop=False,
                        )
                        nc.tensor.matmul(
                            plogs[1][:], g[:, wb + P : wb + 2 * P], xh[:, xs],
                            start=first, stop=False,
                        )
                        # xl @ Wh
                        nc.tensor.matmul(
                            plogs[0][:], g[:, wb : wb + P], xl[:, xs],
                            start=False, stop=False,
                        )
                        nc.tensor.matmul(
                            plogs[1][:], g[:, wb + P : wb + 2 * P], xl[:, xs],
                            start=False, stop=False,
                        )
                        # xh @ Wl
                        nc.tensor.matmul(
                            plogs[0][:], g[:, wb + 2 * P : wb + 3 * P], xh[:, xs],
                            start=False, stop=last,
                        )
                        nc.tensor.matmul(
                            plogs[1][:], g[:, wb + 3 * P : wb + 4 * P], xh[:, xs],
                            start=False, stop=last,
                        )
                    # drip-feed the previous batch's epilogue between chunks
                    if pending and kk >= 1 and kk % 3 == 1:
                        pending.pop(0)()
                while pending:
                    pending.pop(0)()
                pending = epilogue_steps(b, plogs)

            while pending:
                pending.pop(0)()

    nc.compile()
    return nc


def _get_nc(**kw):
    key = tuple(sorted(kw.items()))
    if key not in _CACHE:
        _CACHE[key] = build_program(**kw)
    return _CACHE[key]


def _fp32r_round(a):
    # round-to-nearest fp32 -> fp32r (12-bit mantissa), bit-exact with HW cast
    bits = np.ascontiguousarray(a).view(np.uint32)
    keep = np.uint32(0xFFFFF000)
    rounded = (bits + np.uint32(0x800)) & keep  # round-half-up into kept bits
    # correct round-to-nearest-even on the halfway case
    half = (bits & np.uint32(0xFFF)) == np.uint32(0x800)
    even = ((bits >> np.uint32(12)) & np.uint32(1)) == 0
    rounded = np.where(half & even, bits & keep, rounded)
    return rounded.view(np.float32).reshape(a.shape)


def _pack_x_core(xc):
    """[1024, 7168] fp32 -> [128, 56*1024] d-major, batch-then-chunk packed."""
    parts = []
    t0 = 0
    for TB in BATCHES:
        xb = xc[t0 : t0 + TB]  # [TB, D]
        xb = np.ascontiguousarray(
            xb.reshape(TB, KC, P).transpose(2, 1, 0)
        ).reshape(P, KC * TB)
        parts.append(xb)
        t0 += TB
    return np.ascontiguousarray(np.concatenate(parts, axis=1))


def _pack_w(gate_w):
    """[256, 7168] fp32 -> [128, 56*512] per-chunk [Wh_h0|Wh_h1|Wl_h0|Wl_h1]."""
    gwt = np.ascontiguousarray(gate_w.T)  # [D, E]
    wh = _fp32r_round(gwt)
    wl = gwt - wh  # exact; fits in fp32r
    blocks = []
    for k in range(KC):
        bh = wh[k * P : (k + 1) * P]
        bl = wl[k * P : (k + 1) * P]
        blocks.append(
            np.concatenate(
                [bh[:, :P], bh[:, P:], bl[:, :P], bl[:, P:]], axis=1
            )
        )
    return np.ascontiguousarray(np.concatenate(blocks, axis=1))


def _host_pack(x, gate_w, bias):
    x = np.ascontiguousarray(np.asarray(x, dtype=np.float32))
    gate_w = np.ascontiguousarray(np.asarray(gate_w, dtype=np.float32))
    bias = np.ascontiguousarray(np.asarray(bias, dtype=np.float32))
    gw = _pack_w(gate_w)
    bias2d = bias.reshape(1, E)
    xs = [_pack_x_core(x[c * NPC : (c + 1) * NPC]) for c in range(N_CORES)]
    return xs, gw, bias2d


def _run(x, gate_w, bias, trace=False, **build_kw):
    from concourse.bass_utils import run_bass_kernel_spmd

    nc = _get_nc(**build_kw)
    xs, gw, bias2d = _host_pack(x, gate_w, bias)
    in_maps = [{"x": xs[c], "gw": gw, "bias": bias2d} for c in range(N_CORES)]
    res = run_bass_kernel_spmd(nc, in_maps, core_ids=list(range(N_CORES)), trace=trace)
    idx = np.concatenate([res.results[c]["idx"] for c in range(N_CORES)], axis=0)
    w = np.concatenate([res.results[c]["w"] for c in range(N_CORES)], axis=0)
    return (idx.astype(np.int32), w.astype(np.float32)), res


def kernel(x, gate_w, bias):
    (idx, w), _ = _run(x, gate_w, bias)
    return idx, w


# revision 14
# speedup vs baseline: 1.4923x; 1.1385x over previous
"""DeepSeek-V3 token-choice top-k router on 8 Trainium2 NeuronCores.

Strategy (per core, data-parallel over tokens; 1024 tokens/core):
  - Host pre-transposes x to d-major and packs per k-chunk, so no PE
    transposes are needed: per chunk k the PE computes
    logitsT[e, t] += W_k[d, e].T @ x_k[d, t] with W as the stationary
    operand and tokens streaming (N = batch size columns).
  - Exact fp32 -grade logits via 3 fp32r passes: x split on device into
    hi + lo (exact Sterbenz split, ACT cast + DVE subtract), W split on
    host into hi + lo fp32r.  Accumulated terms: xh@Wh + xl@Wh + xh@Wl
    (dropped xl@Wl is ~2^-26 relative).
  - Tokens processed in batches [512, 256, 256]; each batch's PSUM
    logitsT ([128e, TB] x 2 expert halves) -> ACT sigmoid -> PE
    transpose back to token-major [128t, 256e] -> DVE/ACT routing
    (group top-2 sums, top-4 groups, masked top-8, one-hot gathers,
    normalize).  A batch's routing is emitted interleaved into the next
    batch's chunk loop so only the last (256-token) batch's routing is
    an exposed tail.
  - W DMAs ride the Activation HWDGE queue, the x stream rides the Sync
    queue (2-chunk transfers, ring prefetch).
"""

import numpy as np

N = 8192
D = 7168
E = 256
G = 8
EPG = E // G  # 32
TOPK_GROUP = 4
TOP_K = 8
SCALING = 2.5
N_CORES = 8
NPC = N // N_CORES  # 1024 tokens per core
P = 128
KC = D // P  # 56 contraction chunks
BATCHES = [512, 256, 256]
KU = 2  # k-chunks per DMA/split unit
NKU = KC // KU  # 28 units

_CACHE = {}


def build_program():
    import concourse.bacc as bacc
    import concourse.mybir as mybir
    from concourse import tile, masks

    nc = bacc.Bacc(
        "TRN2",
        target_bir_lowering=False,
        debug=False,
        enable_asserts=True,
        num_devices=N_CORES,
    )
    f32 = mybir.dt.float32
    f32r = mybir.dt.float32r
    i32 = mybir.dt.int32
    u32 = mybir.dt.uint32
    AF = mybir.ActivationFunctionType
    OP = mybir.AluOpType
    AX = mybir.AxisListType

    XCOLS = KC * NPC  # 57344
    x_d = nc.dram_tensor("x", [P, XCOLS], f32, kind="ExternalInput").ap()
    gw_d = nc.dram_tensor("gw", [P, KC * 4 * P], f32r, kind="ExternalInput").ap()
    bias_d = nc.dram_tensor("bias", [1, E], f32, kind="ExternalInput").ap()
    idx_d = nc.dram_tensor("idx", [NPC, TOP_K], i32, kind="ExternalOutput").ap()
    w_d = nc.dram_tensor("w", [NPC, TOP_K], f32, kind="ExternalOutput").ap()

    # column offset of batch b in the packed x layout
    xoff = []
    o = 0
    for TB in BATCHES:
        xoff.append(o)
        o += KC * TB

    with tile.TileContext(nc) as tc:
        with (
            tc.tile_pool(name="const", bufs=1) as const_pool,
            tc.tile_pool(name="gw", bufs=NKU) as gw_pool,
            tc.tile_pool(name="x", bufs=5) as x_pool,
            tc.tile_pool(name="xs", bufs=4) as xs_pool,
            tc.tile_pool(name="plogA", bufs=4, space="PSUM") as plogA_pool,
            tc.tile_pool(name="plogB", bufs=2, space="PSUM") as plogB_pool,
            tc.tile_pool(name="psc", bufs=2, space="PSUM") as psc_pool,
            tc.tile_pool(name="st", bufs=4) as st_pool,
            tc.tile_pool(name="work", bufs=2) as work_pool,
            tc.tile_pool(name="outs", bufs=4) as out_pool,
        ):
            # ---- W rides the ACT HWDGE queue: a few units upfront, the rest
            # ---- dripped inside batch 0's chunk loop (prefetch offset +3)
            # ---- so the 14.7MB doesn't monopolize the queue or the engine.
            gw_tiles = []

            def issue_gw(kk):
                g = gw_pool.tile([P, KU * 4 * P], f32r, tag="gwt", name=f"gw{kk}")
                nc.scalar.dma_start(g[:], gw_d[:, kk * KU * 4 * P : (kk + 1) * KU * 4 * P])
                gw_tiles.append(g)

            for kk in range(4):
                issue_gw(kk)

            bias_sb = const_pool.tile([1, E], f32, name="biassb")
            nc.sync.dma_start(bias_sb[:], bias_d[:])

            # ---- constants ----
            ident = const_pool.tile([P, P], f32)
            masks.make_identity(nc, ident[:])
            iota_i = const_pool.tile([P, E], i32)
            nc.gpsimd.iota(iota_i[:], pattern=[[1, E]], base=0, channel_multiplier=0)
            iota_f = const_pool.tile([P, E], f32)
            nc.vector.tensor_copy(iota_f[:], iota_i[:])
            bias_rep = const_pool.tile([P, E], f32)
            nc.gpsimd.partition_broadcast(bias_rep[:], bias_sb[0:1, :])

            def routing_tile(sfcP, gt0):
                """Route one 128-token tile; sfcP = [128t, 256e] scores in PSUM."""
                sfc = work_pool.tile([P, E], f32, tag="sfc", bufs=3)
                nc.vector.tensor_tensor(sfc[:], sfcP[:], bias_rep[:], op=OP.add)

                # per-group top-8 (need top-2 of each group of 32)
                gtops = work_pool.tile([P, G * 8], f32, tag="gtops")
                for g in range(G):
                    nc.vector.max(
                        gtops[:, g * 8 : (g + 1) * 8],
                        sfc[:, g * EPG : (g + 1) * EPG],
                    )
                gv = gtops[:].rearrange("p (g k) -> p g k", g=G)
                gs = work_pool.tile([P, G], f32, tag="gs")
                nc.vector.tensor_tensor(gs[:], gv[:, :, 0], gv[:, :, 1], op=OP.add)

                # top-4 groups -> mask
                gtop8 = work_pool.tile([P, 8], f32, tag="gtop8")
                nc.vector.max(gtop8[:], gs[:])
                gmask = work_pool.tile([P, G], f32, tag="gmask")
                nc.vector.tensor_scalar(
                    gmask[:], gs[:], gtop8[:, TOPK_GROUP - 1 : TOPK_GROUP], None,
                    op0=OP.is_ge,
                )

                # masked scores
                tmp = work_pool.tile([P, E], f32, tag="tmp")
                for g in range(G):
                    nc.vector.tensor_scalar(
                        tmp[:, g * EPG : (g + 1) * EPG],
                        sfc[:, g * EPG : (g + 1) * EPG],
                        gmask[:, g : g + 1],
                        None,
                        op0=OP.mult,
                    )

                # top-8 values + indices
                vals = work_pool.tile([P, TOP_K], f32, tag="vals")
                nc.vector.max(vals[:], tmp[:])
                idxu = work_pool.tile([P, TOP_K], u32, tag="idxu")
                nc.vector.max_index(idxu[:], vals[:], tmp[:])
                idxf = work_pool.tile([P, TOP_K], f32, tag="idxf")
                nc.vector.tensor_copy(idxf[:], idxu[:])

                # gather raw sigmoid scores at the selected indices
                # (sfcP holds the raw transposed-back sigmoid scores)
                w8 = out_pool.tile([P, TOP_K], f32, tag="w8")
                scratch = work_pool.tile([P, E], f32, tag="scratch")
                for j in range(TOP_K):
                    nc.vector.scalar_tensor_tensor(
                        scratch[:],
                        iota_f[:],
                        idxf[:, j : j + 1],
                        sfcP[:],
                        op0=OP.is_equal,
                        op1=OP.mult,
                        accum_out=w8[:, j : j + 1],
                    )

                # normalize + scale
                wsum = work_pool.tile([P, 1], f32, tag="wsum")
                nc.vector.reduce_sum(wsum[:], w8[:], axis=AX.X)
                wse = work_pool.tile([P, 1], f32, tag="wse")
                nc.vector.tensor_scalar(wse[:], wsum[:], 1e-20, None, op0=OP.add)
                wrec = work_pool.tile([P, 1], f32, tag="wrec")
                nc.vector.reciprocal(wrec[:], wse[:])
                w_out = out_pool.tile([P, TOP_K], f32, tag="wout")
                nc.vector.tensor_scalar(
                    w_out[:], w8[:], wrec[:, 0:1], float(SCALING),
                    op0=OP.mult, op1=OP.mult,
                )
                idx_out = out_pool.tile([P, TOP_K], i32, tag="idxout")
                nc.vector.tensor_copy(idx_out[:], idxu[:])

                nc.sync.dma_start(idx_d[gt0 : gt0 + P, :], idx_out[:])
                nc.sync.dma_start(w_d[gt0 : gt0 + P, :], w_out[:])

            def epilogue_steps(b, pA, pB):
                """Closures: psum-merge + sigmoid head, then one routing/tile
                (each routing step transposes its own token tile first)."""
                TB = BATCHES[b]
                t0 = sum(BATCHES[:b])
                NT = TB // P
                state = {}

                def head():
                    ss = []
                    for h in range(2):
                        if pB is not None:
                            bsb = st_pool.tile([P, TB], f32, tag="bsb", bufs=2,
                                               name=f"bsb{b}{h}")
                            nc.scalar.copy(bsb[:], pB[h][:])
                            lsum = st_pool.tile([P, TB], f32, tag="lsum", bufs=2,
                                                name=f"lsum{b}{h}")
                            nc.vector.tensor_tensor(
                                lsum[:], pA[h][:], bsb[:], op=OP.add
                            )
                            src = lsum
                        else:
                            src = pA[h]
                        s = st_pool.tile([P, TB], f32, tag="sct", name=f"sct{b}{h}")
                        nc.scalar.activation(s[:], src[:], AF.Sigmoid)
                        ss.append(s)
                    state["ss"] = ss

                steps = [head]
                for tt in range(NT):
                    def rt(tt=tt):
                        s0, s1 = state["ss"]
                        sp = psc_pool.tile([P, E], f32, tag="psc")
                        nc.tensor.matmul(
                            sp[:, 0:P], s0[:, tt * P : (tt + 1) * P], ident[:],
                            is_transpose=True,
                        )
                        nc.tensor.matmul(
                            sp[:, P:E], s1[:, tt * P : (tt + 1) * P], ident[:],
                            is_transpose=True,
                        )
                        routing_tile(sp, t0 + tt * P)
                    steps.append(rt)
                return steps

            pending = []  # epilogue closures of the previous batch

            for b, TB in enumerate(BATCHES):
                UC = KU * TB  # columns per x unit
                two_bank = b in (0, 2)
                pA = [
                    plogA_pool.tile([P, TB], f32, tag="plogA", name=f"pA{b}h{h}")
                    for h in range(2)
                ]
                pB = (
                    [
                        plogB_pool.tile([P, TB], f32, tag="plogB", name=f"pB{b}h{h}")
                        for h in range(2)
                    ]
                    if two_bank
                    else None
                )
                pS = pB if two_bank else pA  # where small terms accumulate
                for kk in range(NKU):
                    x_t = x_pool.tile([P, UC], f32, tag="xt", name=f"x{b}_{kk}")
                    nc.sync.dma_start(
                        x_t[:], x_d[:, xoff[b] + kk * UC : xoff[b] + (kk + 1) * UC]
                    )
                    xh = xs_pool.tile([P, UC], f32r, tag="xh")
                    nc.scalar.copy(xh[:], x_t[:])
                    xl = xs_pool.tile([P, UC], f32r, tag="xl")
                    nc.vector.scalar_tensor_tensor(
                        xl[:], x_t[:], 0.0, xh[:].bitcast(f32),
                        op0=OP.add, op1=OP.subtract,
                    )
                    if b == 0 and kk + 4 < NKU:
                        issue_gw(kk + 4)
                    g = gw_tiles[kk]
                    for j in range(KU):
                        k = kk * KU + j
                        wb = j * 4 * P
                        xs = slice(j * TB, (j + 1) * TB)
                        first = k == 0
                        last = k == KC - 1
                        # big terms: xh @ Wh (both halves)
                        nc.tensor.matmul(
                            pA[0][:], g[:, wb : wb + P], xh[:, xs],
                            start=first, stop=(last and two_bank),
                        )
                        nc.tensor.matmul(
                            pA[1][:], g[:, wb + P : wb + 2 * P], xh[:, xs],
                            start=first, stop=(last and two_bank),
                        )
                        # small terms: xl @ Wh + xh @ Wl
                        nc.tensor.matmul(
                            pS[0][:], g[:, wb : wb + P], xl[:, xs],
                            start=(first and two_bank), stop=False,
                        )
                        nc.tensor.matmul(
                            pS[1][:], g[:, wb + P : wb + 2 * P], xl[:, xs],
                            start=(first and two_bank), stop=False,
                        )
                        nc.tensor.matmul(
                            pS[0][:], g[:, wb + 2 * P : wb + 3 * P], xh[:, xs],
                            start=False, stop=last,
                        )
                        nc.tensor.matmul(
                            pS[1][:], g[:, wb + 3 * P : wb + 4 * P], xh[:, xs],
                            start=False, stop=last,
                        )
                    # drip-feed the previous batch's epilogue between chunks
                    if pending and kk >= 1 and kk % 3 == 1:
                        pending.pop(0)()
                while pending:
                    pending.pop(0)()
                pending = epilogue_steps(b, pA, pB)

            while pending:
                pending.pop(0)()

    nc.compile()
    return nc


def _get_nc(**kw):
    key = tuple(sorted(kw.items()))
    if key not in _CACHE:
        _CACHE[key] = build_program(**kw)
    return _CACHE[key]


def _fp32r_round(a):
    # round-to-nearest fp32 -> fp32r (12-bit mantissa), bit-exact with HW cast
    bits = np.ascontiguousarray(a).view(np.uint32)
    keep = np.uint32(0xFFFFF000)
    rounded = (bits + np.uint32(0x800)) & keep  # round-half-up into kept bits
    # correct round-to-nearest-even on the halfway case
    half = (bits & np.uint32(0xFFF)) == np.uint32(0x800)
    even = ((bits >> np.uint32(12)) & np.uint32(1)) == 0
    rounded = np.where(half & even, bits & keep, rounded)
    return rounded.view(np.float32).reshape(a.shape)


def _pack_x_core(xc):
    """[1024, 7168] fp32 -> [128, 56*1024] d-major, batch-then-chunk packed."""
    parts = []
    t0 = 0
    for TB in BATCHES:
        xb = xc[t0 : t0 + TB]  # [TB, D]
        xb = np.ascontiguousarray(
            xb.reshape(TB, KC, P).transpose(2, 1, 0)
        ).reshape(P, KC * TB)
        parts.append(xb)
        t0 += TB
    return np.ascontiguousarray(np.concatenate(parts, axis=1))


def _pack_w(gate_w):
    """[256, 7168] fp32 -> [128, 56*512] per-chunk [Wh_h0|Wh_h1|Wl_h0|Wl_h1]."""
    gwt = np.ascontiguousarray(gate_w.T)  # [D, E]
    wh = _fp32r_round(gwt)
    wl = gwt - wh  # exact; fits in fp32r
    blocks = []
    for k in range(KC):
        bh = wh[k * P : (k + 1) * P]
        bl = wl[k * P : (k + 1) * P]
        blocks.append(
            np.concatenate(
                [bh[:, :P], bh[:, P:], bl[:, :P], bl[:, P:]], axis=1
            )
        )
    return np.ascontiguousarray(np.concatenate(blocks, axis=1))


def _host_pack(x, gate_w, bias):
    x = np.ascontiguousarray(np.asarray(x, dtype=np.float32))
    gate_w = np.ascontiguousarray(np.asarray(gate_w, dtype=np.float32))
    bias = np.ascontiguousarray(np.asarray(bias, dtype=np.float32))
    gw = _pack_w(gate_w)
    bias2d = bias.reshape(1, E)
    xs = [_pack_x_core(x[c * NPC : (c + 1) * NPC]) for c in range(N_CORES)]
    return xs, gw, bias2d


def _run(x, gate_w, bias, trace=False, **build_kw):
    from concourse.bass_utils import run_bass_kernel_spmd

    nc = _get_nc(**build_kw)
    xs, gw, bias2d = _host_pack(x, gate_w, bias)
    in_maps = [{"x": xs[c], "gw": gw, "bias": bias2d} for c in range(N_CORES)]
    res = run_bass_kernel_spmd(nc, in_maps, core_ids=list(range(N_CORES)), trace=trace)
    idx = np.concatenate([res.results[c]["idx"] for c in range(N_CORES)], axis=0)
    w = np.concatenate([res.results[c]["w"] for c in range(N_CORES)], axis=0)
    return (idx.astype(np.int32), w.astype(np.float32)), res


def kernel(x, gate_w, bias):
    (idx, w), _ = _run(x, gate_w, bias)
    return idx, w
